# revision 29
# baseline (speedup 1.0000x reference)
"""GroupedQueryAttention on 8 trn2 NeuronCores.

Full shapes: q [2,8,4,2048,128], k/v [2,8,1,2048,128] -> out [2,8,4,2048,128]
softmax over S (no 1/sqrt(D) scaling; constant -64 shift keeps exp in range).

The end-to-end time here is dominated by the axon-tunneled host<->device
transfers, not device compute, so the warm path is engineered around the
wire:
  - q/k ship as fp16 (scores need the mantissa — bf16 q/k fails the absmax
    gate; |scores| <~ 100 so fp16 range is fine). v ships as per-s-row
    symmetric int8 + fp16 scales (its quantization error flows through a
    convex softmax combination, no exp amplification) and is dequantized
    to bf16 on-chip before the PV matmul.
  - q/k/v are packed into ONE uint16 dram tensor (the tunnel charges ~100ms
    of fixed overhead per transferred array); the kernel bitcasts slices.
  - output returns as per-row symmetric int8 with the row's fp16 scale
    packed into 2 extra bytes ([NH*L, 130] int8): halves d2h AND the
    output-sized donated-zeros upload the axon path makes every call.
    Host dequantizes (absmax/scale cost ~4e-3, total 7.9e-3 vs 2e-2 gate).
  - jax's persistent compilation cache is enabled: run_bass_kernel_spmd
    re-creates its jax.jit closure every call, and without the cache each
    warm call re-runs lower+walrus+load (~0.3s).

Sharding: 16 (b,h) kv pairs across 8 cores -> 2 pairs/core, each pair has
G=4 query heads sharing one K/V. Per core: 8 independent 2048x2048x128
attention heads, no communication.

Per-core kernel (all matmuls contract over the 128-partition dim):
  - K^T, Q^T prepared via PE transposes (fp16).
  - scoresT [s_tile=128, l_chunk=512] = KT.T @ QT (fp16 in, fp32 PSUM)
  - ACT evicts PSUM->SBUF with Exp, output bf16 (exp(s-64) can reach ~1e15,
    needs bf16 range; bf16 probs already proven within the error budget).
  - PV: outT [d=128, l=512] += V.T-form matmul (lhsT=V natural, bf16)
  - softmax denominator: DVE bf16 adds (2x mode) over the 16 exp tiles
    (2 split accumulators to shorten the bf16 rounding chain), then GPSIMD
    partition_all_reduce across the s-partitions.
  - normalize outT with DVE reciprocal+mul, PE-transpose back to natural
    [l,d] layout, then per-row abs-max (DVE reduce) -> fp16 scale, ACT
    Copy-with-scale quantizes to int8, DMA out data+scale rows.
"""

import numpy as np

D = 128
L = 2048
S = 2048
G = 4  # query heads per kv head
NP = 2  # kv pairs per core
NH = NP * G  # 8 q-heads per core
LC = 512  # l chunk (matmul moving free dim)
NLC = L // LC  # 4
NST = S // 128  # 16 s tiles
NLT = L // 128  # 16 l tiles
NCORES = 8

# packed per-pair row layout in the single uint16 input tensor:
# rows [0, QB) = q (4 heads x 2048, fp16 bits), [QB, KB) = k (fp16 bits),
# then v as per-s-row symmetric int8 (S/2 uint16-rows of data, s-row s at
# u16-row s//2 bytes (s%2)*128..) followed by NST rows of fp16 scales
# (scale for s-row s at row s//128, col s%128)
QB = G * L  # 8192
KB = QB + S  # 10240
VD = KB  # v int8 data start
VS = VD + S // 2  # 11264, v scales start
PAIR_ROWS = VS + NST  # 11280

_CACHE = {}


def _build_nc():
    import concourse.bass as bass
    import concourse.bacc as bacc
    import concourse.bass_isa as bass_isa
    import concourse.mybir as mybir
    import concourse.tile as tile
    from concourse.masks import make_identity

    f32 = mybir.dt.float32
    f16 = mybir.dt.float16
    bf16 = mybir.dt.bfloat16
    u16 = mybir.dt.uint16
    i8 = mybir.dt.int8
    AF = mybir.ActivationFunctionType
    ALU = mybir.AluOpType

    nc = bacc.Bacc("TRN2")
    x = nc.declare_dram_parameter("x", [NP, PAIR_ROWS, D], u16, isOutput=False)
    # output rows: 128 int8 quantized values + the row's fp16 scale as 2 bytes
    o = nc.declare_dram_parameter("o", [NH * L, D + 2], i8, isOutput=True)

    with tile.TileContext(nc) as tc:
        with (
            tc.tile_pool(name="const", bufs=1) as constp,
            tc.tile_pool(name="kt", bufs=2) as ktp,
            tc.tile_pool(name="qt", bufs=2) as qtp,
            tc.tile_pool(name="vv", bufs=2) as vvp,
            tc.tile_pool(name="nat", bufs=4) as natp,
            tc.tile_pool(name="pe", bufs=10) as pep,
            tc.tile_pool(name="acc", bufs=16) as accp,
            tc.tile_pool(name="epi", bufs=8) as epip,
            tc.tile_pool(name="onat", bufs=12) as onatp,
            tc.tile_pool(name="psum", bufs=4, space="PSUM") as psump,
        ):
            identh = constp.tile([128, 128], f16, tag="identh")
            make_identity(nc, identh)
            identf = constp.tile([128, 128], f32, tag="identf")
            make_identity(nc, identf)
            nbias = constp.tile([128, 1], f32, tag="nbias")
            nc.vector.memset(nbias, -64.0)
            ones = constp.tile([128, 128], f32, tag="ones")
            nc.vector.memset(ones, 1.0)

            for pair in range(NP):
                # ---- K^T [d=128, S] via PE transposes (fp16) ----
                KT = ktp.tile([128, S], f16, tag="KT")
                for st in range(NST):
                    knat = natp.tile([128, D], f16, tag="knat")
                    nc.sync.dma_start(
                        out=knat,
                        in_=x[pair, QB + st * 128 : QB + (st + 1) * 128, :].bitcast(
                            f16
                        ),
                    )
                    pt = psump.tile([128, 128], f16, tag="ps")
                    nc.tensor.transpose(pt, knat, identh)
                    nc.vector.tensor_copy(KT[:, st * 128 : (st + 1) * 128], pt)

                # ---- V: int8 data + fp16 per-s-row scales -> dequant to bf16 ----
                scT = vvp.tile([128, NST], f16, tag="scT")
                nc.sync.dma_start(
                    out=scT,
                    in_=x[pair, VS : VS + NST, :].bitcast(f16).rearrange("t p -> p t"),
                )
                scF = vvp.tile([128, NST], f32, tag="scF")
                nc.vector.tensor_copy(scF, scT)
                Vq = vvp.tile([128, NST, D], i8, tag="Vq")
                nc.sync.dma_start(
                    out=Vq,
                    in_=x[pair, VD : VD + S // 2, :]
                    .bitcast(i8)
                    .rearrange(
                        "(t h2) (pp d) -> (h2 pp) t d", t=NST, h2=64, pp=2, d=D
                    ),
                )
                Vb = vvp.tile([128, NST, D], bf16, tag="Vb")
                for t in range(NST):
                    nc.vector.tensor_copy(Vb[:, t, :], Vq[:, t, :])
                    nc.vector.tensor_scalar_mul(
                        Vb[:, t, :], Vb[:, t, :], scF[:, t : t + 1]
                    )

                for g in range(G):
                    h = pair * G + g
                    # ---- Q^T [d=128, L] via PE transposes (fp16) ----
                    QT = qtp.tile([128, L], f16, tag="QT")
                    qrow = g * L
                    for lt in range(NLT):
                        qnat = natp.tile([128, D], f16, tag="qnat")
                        nc.sync.dma_start(
                            out=qnat,
                            in_=x[
                                pair, qrow + lt * 128 : qrow + (lt + 1) * 128, :
                            ].bitcast(f16),
                        )
                        pt = psump.tile([128, 128], f16, tag="ps")
                        nc.tensor.transpose(pt, qnat, identh)
                        nc.vector.tensor_copy(QT[:, lt * 128 : (lt + 1) * 128], pt)

                    # out^T accumulators, one PSUM bank per l-chunk
                    po = [
                        psump.tile([128, LC], f32, tag="po", name=f"po_{h}_{lc}")
                        for lc in range(NLC)
                    ]
                    # split bf16 denominator accumulators (even/odd st)
                    acc = [
                        [
                            accp.tile(
                                [128, LC], bf16, tag="acc", name=f"acc_{h}_{lc}_{i}"
                            )
                            for i in range(2)
                        ]
                        for lc in range(NLC)
                    ]

                    for st in range(NST):
                        pss = []
                        for lc in range(NLC):
                            ps = psump.tile([128, LC], f32, tag="ps")
                            nc.tensor.matmul(
                                ps,
                                lhsT=KT[:, st * 128 : (st + 1) * 128],
                                rhs=QT[:, lc * LC : (lc + 1) * LC],
                                start=True,
                                stop=True,
                            )
                            pss.append(ps)
                        for lc in range(NLC):
                            pe = pep.tile([128, LC], bf16, tag="pe")
                            # exp(s - 64): constant shift keeps exp in fp32/bf16
                            # range (scores reach ~99; fp32 exp overflows at 88)
                            nc.scalar.activation(pe, pss[lc], AF.Exp, bias=nbias)
                            nc.tensor.matmul(
                                po[lc],
                                lhsT=Vb[:, st, :],
                                rhs=pe,
                                start=(st == 0),
                                stop=(st == NST - 1),
                            )
                            a = acc[lc][st % 2]
                            if st < 2:
                                nc.vector.tensor_copy(a, pe)
                            else:
                                nc.vector.tensor_tensor(
                                    out=a, in0=a, in1=pe, op=ALU.add
                                )

                    for lc in range(NLC):
                        den = epip.tile([128, LC], f32, tag="den")
                        nc.vector.tensor_tensor(
                            out=den, in0=acc[lc][0], in1=acc[lc][1], op=ALU.add
                        )
                        # partition all-reduce via PE ones-matmul: every output
                        # partition gets sum_p den[p, l]. ~1.5us on PE vs
                        # multiple ms per gpsimd.partition_all_reduce call.
                        psd = psump.tile([128, LC], f32, tag="ps")
                        nc.tensor.matmul(
                            psd, lhsT=ones, rhs=den, start=True, stop=True
                        )
                        rec = epip.tile([128, LC], f32, tag="rec")
                        nc.vector.reciprocal(rec, psd)
                        oT = epip.tile([128, LC], f32, tag="oT")
                        nc.vector.tensor_tensor(
                            out=oT, in0=po[lc], in1=rec, op=ALU.mult
                        )
                        for j in range(4):
                            ptr = psump.tile([128, 128], f32, tag="ps")
                            nc.tensor.transpose(
                                ptr, oT[:, j * 128 : (j + 1) * 128], identf
                            )
                            # per-row (per-l) symmetric int8 quantization:
                            # ship fp16 scale = absmax/127 in the last 2 bytes
                            amax = epip.tile([128, 1], f32, tag="amax")
                            nc.vector.tensor_reduce(
                                amax,
                                ptr,
                                mybir.AxisListType.X,
                                ALU.max,
                                apply_absolute_value=True,
                            )
                            s16 = epip.tile([128, 1], f16, tag="s16")
                            nc.vector.tensor_scalar_mul(s16, amax, 1.0 / 127.0)
                            rinv = epip.tile([128, 1], f32, tag="rinv")
                            nc.vector.reciprocal(rinv, amax)
                            rinv127 = epip.tile([128, 1], f32, tag="rinv127")
                            nc.vector.tensor_scalar_mul(rinv127, rinv, 127.0)
                            onat = onatp.tile([128, D + 2], i8, tag="onat")
                            nc.scalar.mul(onat[:, 0:D], ptr, rinv127)
                            nc.vector.tensor_copy(
                                onat[:, D : D + 2], s16.bitcast(i8)
                            )
                            lt = lc * 4 + j
                            r0 = h * L + lt * 128
                            nc.sync.dma_start(out=o[r0 : r0 + 128, :], in_=onat)
    if not nc.is_finalized():
        nc.finalize()
    return nc


def _get_nc():
    if "nc" not in _CACHE:
        _CACHE["nc"] = _build_nc()
    return _CACHE["nc"]


def _enable_compile_cache():
    # run_bass_kernel_spmd re-creates its jax.jit closure every call, so
    # without a persistent cache each warm call re-runs lower+walrus+load
    # (~0.3s). The axon PJRT supports executable (de)serialization, so the
    # disk cache turns that into a hash lookup.
    if "cc" in _CACHE:
        return
    _CACHE["cc"] = True
    try:
        import jax

        jax.config.update("jax_compilation_cache_dir", "/tmp/jax_pjrt_cache")
        jax.config.update("jax_persistent_cache_min_entry_size_bytes", -1)
        jax.config.update("jax_persistent_cache_min_compile_time_secs", 0)
        jax.config.update("jax_persistent_cache_enable_xla_caches", "all")
    except Exception:
        pass


def _cpu_fns():
    # fused multithreaded cast/pack + dequant on the XLA CPU backend
    # (~3x faster than single-threaded numpy); falls back to numpy if the
    # cpu platform is unavailable
    if "cpu_fns" in _CACHE:
        return _CACHE["cpu_fns"]
    try:
        import jax
        import jax.numpy as jnp
        from jax import lax

        cpu = jax.devices("cpu")[0]

        def pack(qf, kf, vf):
            xq = lax.bitcast_convert_type(qf.astype(jnp.float16), jnp.uint16)
            xk = lax.bitcast_convert_type(kf.astype(jnp.float16), jnp.uint16)
            amax = jnp.max(jnp.abs(vf), axis=-1, keepdims=True)  # [16,S,1]
            sc = (amax / 127.0).astype(jnp.float16)
            inv = 1.0 / jnp.where(sc == 0, jnp.float16(1), sc).astype(jnp.float32)
            vq = jnp.clip(jnp.round(vf * inv), -127, 127).astype(jnp.int8)
            xv = lax.bitcast_convert_type(
                vq.reshape(16, S, D // 2, 2), jnp.uint16
            ).reshape(16, S // 2, D)
            xs = lax.bitcast_convert_type(sc[:, :, 0], jnp.uint16).reshape(
                16, NST, D
            )
            return jnp.concatenate([xq, xk, xv, xs], axis=1)

        def dequant(*os):  # 8x [NH*L, D+2] int8, core order
            r = jnp.stack(os)  # [8, NH*L, D+2]
            sc = lax.bitcast_convert_type(r[:, :, D : D + 2], jnp.float16)
            out = r[:, :, :D].astype(jnp.float32) * sc[..., None].astype(
                jnp.float32
            )
            return out.reshape(16, G, L, D)

        # hold AOT-compiled executables: they survive the per-call
        # jax.clear_caches() (no re-trace / re-load each call)
        f32a = jax.ShapeDtypeStruct
        fns = {
            "pack": jax.jit(pack, device=cpu)
            .lower(
                f32a((16, QB, D), np.float32),
                f32a((16, S, D), np.float32),
                f32a((16, S, D), np.float32),
            )
            .compile(),
            "dequant": jax.jit(dequant, device=cpu)
            .lower(*[f32a((NH * L, D + 2), np.int8)] * NCORES)
            .compile(),
        }
    except Exception:
        fns = None
    _CACHE["cpu_fns"] = fns
    return fns


def _run(q, k, v, trace=False, trace_kwargs=None):
    import ml_dtypes
    from concourse.bass_utils import run_bass_kernel_spmd

    _enable_compile_cache()
    nc = _get_nc()
    q = np.asarray(q)
    k = np.asarray(k)
    v = np.asarray(v)
    # (b,h) pair index = b*8+h; core c owns pairs 2c, 2c+1.
    # Pack q(fp16) / k(fp16) / v(bf16) bit patterns into one uint16 tensor.
    fns = _cpu_fns()
    xfull = None
    if fns is not None:
        try:
            xfull = np.asarray(
                fns["pack"](
                    q.reshape(16, QB, D).astype(np.float32, copy=False),
                    k.reshape(16, S, D).astype(np.float32, copy=False),
                    v.reshape(16, S, D).astype(np.float32, copy=False),
                )
            )
        except Exception:
            xfull = None
    if xfull is None:
        xfull = np.empty((16, PAIR_ROWS, D), np.uint16)
        np.copyto(
            xfull[:, :QB, :].view(np.float16), q.reshape(16, QB, D), casting="unsafe"
        )
        np.copyto(
            xfull[:, QB:KB, :].view(np.float16), k.reshape(16, S, D), casting="unsafe"
        )
        vsq = np.asarray(v, np.float32).reshape(16, S, D)
        amax = np.abs(vsq).max(axis=-1, keepdims=True)
        sc = (amax / 127.0).astype(np.float16)
        inv = 1.0 / np.where(sc == 0, np.float16(1), sc).astype(np.float32)
        vq = np.clip(np.rint(vsq * inv), -127, 127).astype(np.int8)
        xfull[:, VD:VS, :] = vq.reshape(16, S // 2, 2 * D).view(np.uint16)
        xfull[:, VS:, :] = sc[:, :, 0].view(np.uint16).reshape(16, NST, D)
    in_maps = [{"x": xfull[2 * c : 2 * c + 2]} for c in range(NCORES)]
    kwargs = {}
    if trace:
        kwargs["trace"] = True
        if trace_kwargs:
            kwargs.update(trace_kwargs)
    # transient tunnel/device errors (e.g. NRT_EXEC_UNIT_UNRECOVERABLE) have
    # been observed; one clean-slate retry is cheap and sometimes recovers
    try:
        res = run_bass_kernel_spmd(nc, in_maps, list(range(NCORES)), **kwargs)
    except Exception:
        import gc
        import time

        import jax

        jax.clear_caches()
        gc.collect()
        time.sleep(2.0)
        res = run_bass_kernel_spmd(nc, in_maps, list(range(NCORES)), **kwargs)
    # gather + dequantize: out = int8 data * per-row fp16 scale
    out = None
    if fns is not None:
        try:
            out = np.asarray(
                fns["dequant"](*[res.results[c]["o"] for c in range(NCORES)])
            )
        except Exception:
            out = None
    if out is None:
        out = np.empty((16, G, L, D), dtype=np.float32)
        for c in range(NCORES):
            r = res.results[c]["o"].reshape(NH, L, D + 2)
            sc = (
                np.ascontiguousarray(r[:, :, D : D + 2])
                .view(np.float16)
                .astype(np.float32)
            )  # [NH, L, 1]
            np.multiply(
                r[:, :, :D],
                sc,
                out=out[2 * c : 2 * c + 2].reshape(NH, L, D),
                casting="unsafe",
            )
    # run_bass_kernel_spmd makes a fresh jit closure per call, so its cache
    # entries are dead on return; left around they accumulate and slow later
    # calls by ~1s. Dropping them costs ~0.08s.
    try:
        import gc

        import jax

        jax.clear_caches()
        # gen-1 frees this call's cyclic jit garbage at ~1/10 the cost of a
        # full pass; a periodic full pass catches anything promoted to gen-2
        n = _CACHE["calls"] = _CACHE.get("calls", 0) + 1
        gc.collect() if n % 8 == 0 else gc.collect(1)
    except Exception:
        pass
    return out.reshape(2, 8, G, L, D), res


def kernel(q, k, v):
    out, _ = _run(q, k, v, trace=False)
    return out
